# revision 1
# baseline (speedup 1.0000x reference)
"""Multi-head attention with LoRA adapters on 8 Trainium2 NeuronCores.

Problem: x[4,2048,768] -> LoRA-linear QKV -> 12-head attention -> LoRA-linear out proj.

Math notes:
  - LoRA is folded into the base weights on the host:  x@W.T + b + (x@A.T)@B.T
    == x@(W + B@A).T + b  (exact up to fp rounding).
  - The value bias bv is folded into the output bias using softmax(row)@1 == 1:
    (attn@(v + bv)) @ Wo.T + bo == (attn@v)@Wo.T + (bo + Wo@bv).
  - Softmax is computed without max-subtraction (scores are O(+-10), safe in fp32);
    the row sum is obtained by augmenting v with a ones column, and the division
    is applied to the (tiny) attention output rather than the attention matrix.
  - Matmuls run in float32r (fp32 with 11-bit mantissa, full PE rate at free
    dim >= 256). Operands are rounded to f32r by their producing engine op, or
    on the host for DMA-direct inputs.

Sharding: core = 2*b + g for batch b in 0..3, head-group g in 0..1 (6 heads each).
Each core computes its 6 heads' attention output and a partial output projection
(row-sharded Wo); host sums the two partials per batch.

Layouts per core (DIM=768, CS=384 local channels, T=2048):
  xT   [768, 2048]   x[b] transposed (host-prepared)
  qT,kT[384, 2048]   projections in [channel, time] layout (3 sbuf blocks of 128)
  v    [2048, 390]   natural [time, channel] layout, 6 heads x (64 ch + 1 ones col)
  scores transposed: S^T[s, t] = sum_hd kT[hd,s] * qT[hd,t]  (PE, K=64)
  PV:   out^T[c, t] = sum_s v_aug[s, c] * exp(S^T)[s, t]     (PE, K=128, M=65)
  proj: out[t, d]  = sum_c outT[c, t] * woT[c, d]            (PE, K=128)

Schedule notes (the PE runs its queue in order, so emission order matters):
  - QKV projections for channel block cb are emitted just before the attention
    of heads 2cb/2cb+1; one shared PSUM pool lets projection, score and output
    matmul groups pipeline through the same two 2-bank slots.
  - PV matmuls are emitted one s-step behind the score matmuls so the PE never
    waits for exp(s) on the Scalar engine.
"""

import contextlib
import os
import sys

sys.path.insert(0, "/opt/trn_rl_repo")

import numpy as np

# Wrap the whole kernel body in a hardware loop (timing harness only; the
# graded path uses REPEAT=1).
REPEAT = int(os.environ.get("KERNEL_REPEAT", "1"))
# bf16 q/k: halves their SBUF and enables fast weight load on score matmuls
BF16_QK = os.environ.get("KERNEL_BF16_QK", "1") == "1"

DIM, HEADS, R = 768, 12, 8
B, T = 4, 2048
HD = DIM // HEADS          # 64 head dim
NCORES = 8
HG = HEADS // 2            # 6 heads per core
CS = HG * HD               # 384 local channels per core
SCALE = HD ** -0.5

_PROGRAM_CACHE = {}


def _bf16(a):
    import ml_dtypes
    return np.ascontiguousarray(a).astype(ml_dtypes.bfloat16)


def _round_f32r(a):
    """Round fp32 to f32r (11-bit mantissa) with round-to-nearest-even."""
    u = np.ascontiguousarray(a, dtype=np.float32).view(np.uint32)
    r = (u + np.uint32(0x7FF) + ((u >> np.uint32(12)) & np.uint32(1))) & np.uint32(0xFFFFF000)
    return r.view(np.float32)


def _build_program():
    import concourse.bass as bass
    import concourse.mybir as mybir
    import concourse.tile as tile
    from concourse import bacc

    f32 = mybir.dt.float32
    f32r = mybir.dt.float32r
    qk_dt = mybir.dt.bfloat16 if BF16_QK else f32r

    nc = bacc.Bacc("TRN2", target_bir_lowering=False, debug=False,
                   num_devices=NCORES)

    bf16 = mybir.dt.bfloat16
    xT = nc.dram_tensor("xT", [DIM, T], bf16, kind="ExternalInput")
    wq_t = nc.dram_tensor("wq_t", [DIM, CS], bf16, kind="ExternalInput")
    wk_t = nc.dram_tensor("wk_t", [DIM, CS], bf16, kind="ExternalInput")
    wv_t = nc.dram_tensor("wv_t", [DIM, CS], bf16, kind="ExternalInput")
    wo_t = nc.dram_tensor("wo_t", [CS, DIM], bf16, kind="ExternalInput")
    bq_s = nc.dram_tensor("bq_s", [CS], f32, kind="ExternalInput")
    bk_s = nc.dram_tensor("bk_s", [CS], f32, kind="ExternalInput")
    bo_s = nc.dram_tensor("bo_s", [DIM], f32, kind="ExternalInput")
    out_p = nc.dram_tensor("out_p", [T, DIM], f32, kind="ExternalOutput")

    KB = DIM // 128      # 6 k-blocks of the input dim
    CB = CS // 128       # 3 channel blocks (head pairs)
    TB = T // 128        # 16 time tiles
    TJ = T // 1024       # 2 big time chunks for the attention loop
    VW = HD + 1          # 65: v plus ones column

    with tile.TileContext(nc) as tc:
        with (
            tc.tile_pool(name="weights", bufs=1) as wpool,
            tc.tile_pool(name="persist", bufs=1) as ppool,
            tc.tile_pool(name="ps", bufs=2, space="PSUM") as ps_pool,
            tc.tile_pool(name="psC", bufs=2, space="PSUM") as psC_pool,
            tc.tile_pool(name="epool", bufs=5) as e_pool,
            tc.tile_pool(name="npool", bufs=2) as n_pool,
            tc.tile_pool(name="rspool", bufs=2, space="DRAM") as rs_pool,
            (tc.For_i(0, REPEAT, 1) if REPEAT > 1 else contextlib.nullcontext()),
        ):
            # ---- input loads (w_ot/bo deferred until phase D) ----
            w_kt = wpool.tile([128, KB, CS], bf16)
            nc.sync.dma_start(out=w_kt, in_=wk_t.ap().rearrange("(k p) m -> p k m", p=128))
            w_qt = wpool.tile([128, KB, CS], bf16)
            nc.sync.dma_start(out=w_qt, in_=wq_t.ap().rearrange("(k p) m -> p k m", p=128))
            xT_sb = wpool.tile([128, KB, T], bf16)
            xT_view = xT.ap().rearrange("(k p) t -> p k t", p=128)
            for kk in range(KB):
                nc.sync.dma_start(out=xT_sb[:, kk, :], in_=xT_view[:, kk, :])
            w_vt = wpool.tile([128, KB, CS], bf16)
            nc.sync.dma_start(out=w_vt, in_=wv_t.ap().rearrange("(k p) m -> p k m", p=128))
            bq_sb = wpool.tile([128, CB], f32)
            nc.sync.dma_start(out=bq_sb, in_=bq_s.ap().rearrange("(k p) -> p k", p=128))
            bk_sb = wpool.tile([128, CB], f32)
            nc.sync.dma_start(out=bk_sb, in_=bk_s.ap().rearrange("(k p) -> p k", p=128))

            # ---- persistent activations ----
            qT_sb = ppool.tile([128, CB, T], qk_dt)
            kT_sb = ppool.tile([128, CB, T], qk_dt)
            v_sb = ppool.tile([128, TB, HG * VW], f32r)
            outT_sb = ppool.tile([128, CB, T], bf16)

            # ones columns of v_aug (one strided memset covers all 16x6 columns)
            ones_ap = bass.AP(
                tensor=v_sb.tensor, offset=v_sb.offset + HD,
                ap=[v_sb.ap[0], [HG * VW, TB], [VW, HG]],
            )
            nc.vector.memset(ones_ap.bitcast(mybir.dt.uint32), 0x3F800000)

            def proj_qk(dst, w, bias, cb):
                # one channel block of the q^T / k^T projection
                for jt in range(T // 512):
                    psp = ps_pool.tile([128, 512], f32, tag="ps")
                    for kk in range(KB):
                        nc.tensor.matmul(
                            psp,
                            w[:, kk, cb * 128:(cb + 1) * 128],
                            xT_sb[:, kk, jt * 512:(jt + 1) * 512],
                            start=(kk == 0), stop=(kk == KB - 1),
                        )
                    nc.vector.tensor_scalar_add(
                        dst[:, cb, jt * 512:(jt + 1) * 512], psp,
                        bias[:, cb:cb + 1],
                    )

            def proj_v():
                for st in range(TB):
                    psp = ps_pool.tile([128, CS], f32, tag="ps")
                    for kk in range(KB):
                        nc.tensor.matmul(
                            psp,
                            xT_sb[:, kk, st * 128:(st + 1) * 128],
                            w_vt[:, kk, :],
                            start=(kk == 0), stop=(kk == KB - 1),
                        )
                    # scatter 6 heads' 64 channels into the 65-wide slots
                    src = psp.rearrange("p (h c) -> p h c", h=HG)
                    dst = bass.AP(
                        tensor=v_sb.tensor,
                        offset=v_sb.offset + st * (HG * VW),
                        ap=[v_sb.ap[0], [VW, HG], [1, HD]],
                    )
                    nc.vector.tensor_copy(dst, src)

            def attention(h):
                hb, hblk = h % 2, h // 2
                prow = slice(64 * hb, 64 * hb + 64)
                for j in range(TJ):
                    psC = psC_pool.tile([VW, 1024], f32, tag="psC")

                    def emit_c(s, e):
                        # PV matmul for step s, emitted one step late so the
                        # in-order PE queue never waits on exp(s)
                        lhs_v = v_sb[:, s, h * VW:(h + 1) * VW]
                        for half in range(2):
                            cols = slice(half * 512, half * 512 + 512)
                            nc.tensor.matmul(
                                psC[:, cols],
                                lhs_v,
                                e[:, cols],
                                start=(s == 0), stop=(s == TB - 1),
                            )

                    pending = None
                    for s in range(TB):
                        psB = ps_pool.tile([128, 1024], f32, tag="ps")
                        lhs_k = kT_sb[prow, hblk, s * 128:(s + 1) * 128]
                        for half in range(2):
                            cols = slice(half * 512, half * 512 + 512)
                            tcols = slice(j * 1024 + half * 512,
                                          j * 1024 + half * 512 + 512)
                            nc.tensor.matmul(
                                psB[:, cols],
                                lhs_k,
                                qT_sb[prow, hblk, tcols],
                                start=True, stop=True,
                            )
                        e = e_pool.tile([128, 1024], f32r, tag="e")
                        nc.scalar.activation(
                            e, psB, mybir.ActivationFunctionType.Exp,
                            scale=SCALE,
                        )
                        if pending is not None:
                            emit_c(*pending)
                        pending = (s, e)
                    emit_c(*pending)

                    # normalize: rows 0..63 are sum(e*v), row 64 is sum(e).
                    # one [65,1024] scratch tile: reciprocal into row 64, then
                    # partition-broadcast into rows 0..63 via a DRAM bounce
                    # (step-0 partition DMA; gpsimd partition_broadcast reads
                    # the wrong partition for base!=0 sources on HW), multiply
                    # in place (f32r out)
                    nr = n_pool.tile([65, 1024], f32, tag="nr")
                    nc.vector.reciprocal(nr[HD:VW, :], psC[HD:VW, :])
                    rs = rs_pool.tile([1, 1024], f32, tag="rs")
                    nc.sync.dma_start(out=rs, in_=nr[HD:VW, :])
                    rs_bcast = bass.AP(tensor=rs.tensor, offset=rs.offset,
                                       ap=[[0, HD], [1, 1024]])
                    nc.sync.dma_start(out=nr[0:HD, :], in_=rs_bcast)
                    onorm = n_pool.tile([64, 1024], bf16, tag="onorm")
                    nc.vector.tensor_mul(onorm, psC[0:HD, :], nr[0:HD, :])
                    # place into the [c, t] pair-block layout (partition shift
                    # for odd heads -> SBUF-to-SBUF DMA)
                    nc.sync.dma_start(
                        out=outT_sb[prow, hblk, j * 1024:(j + 1) * 1024],
                        in_=onorm,
                    )

            # ---- interleaved projection + attention, per channel block ----
            for cb in range(CB):
                proj_qk(kT_sb, w_kt, bk_sb, cb)
                proj_qk(qT_sb, w_qt, bq_sb, cb)
                if cb == 0:
                    proj_v()
                attention(2 * cb)
                attention(2 * cb + 1)

            # ---- phase D: output projection (row-sharded) + bias ----
            w_ot = wpool.tile([128, CB, DIM], bf16)
            nc.sync.dma_start(out=w_ot, in_=wo_t.ap().rearrange("(k p) m -> p k m", p=128))
            bo_row = wpool.tile([1, DIM], f32)
            nc.sync.dma_start(out=bo_row, in_=bo_s.ap().rearrange("(o d) -> o d", o=1))
            bo_sb = wpool.tile([128, DIM], f32)
            nc.gpsimd.partition_broadcast(bo_sb, bo_row)
            for mt in range(TB):
                psD = ps_pool.tile([128, DIM], f32, tag="ps")
                for ncols in (slice(0, 512), slice(512, DIM)):
                    for cb in range(CB):
                        nc.tensor.matmul(
                            psD[:, ncols],
                            outT_sb[:, cb, mt * 128:(mt + 1) * 128],
                            w_ot[:, cb, ncols],
                            start=(cb == 0), stop=(cb == CB - 1),
                        )
                osb = e_pool.tile([128, DIM], f32, tag="osb")
                nc.vector.tensor_add(osb, psD, bo_sb)
                nc.sync.dma_start(
                    out=out_p.ap()[mt * 128:(mt + 1) * 128, :], in_=osb,
                )

    nc.compile()
    return nc


def _get_program():
    if "nc" not in _PROGRAM_CACHE:
        _PROGRAM_CACHE["nc"] = _build_program()
    return _PROGRAM_CACHE["nc"]


def _prep_in_maps(inputs):
    f = np.float32

    def eff(w, a, bl):
        return (w.astype(np.float64) + bl.astype(np.float64) @ a.astype(np.float64)).astype(f)

    wq = eff(inputs["wq"], inputs["laq"], inputs["lbq"])
    wk = eff(inputs["wk"], inputs["lak"], inputs["lbk"])
    wv = eff(inputs["wv"], inputs["lav"], inputs["lbv"])
    wo = eff(inputs["wo"], inputs["lao"], inputs["lbo"])
    x = np.asarray(inputs["x"], dtype=f)
    bq, bk, bv, bo = (np.asarray(inputs[k], dtype=f) for k in ("bq", "bk", "bv", "bo"))

    in_maps = []
    for core in range(NCORES):
        b, g = core // 2, core % 2
        cols = slice(g * CS, (g + 1) * CS)
        bo_core = wo[:, cols].astype(np.float64) @ bv[cols].astype(np.float64)
        if g == 0:
            bo_core = bo_core + bo
        in_maps.append({
            "xT": _bf16(x[b].T),
            "wq_t": _bf16(wq[cols, :].T),
            "wk_t": _bf16(wk[cols, :].T),
            "wv_t": _bf16(wv[cols, :].T),
            "wo_t": _bf16(wo[:, cols].T),
            "bq_s": np.ascontiguousarray(bq[cols]),
            "bk_s": np.ascontiguousarray(bk[cols]),
            "bo_s": bo_core.astype(f),
        })
    return in_maps


def kernel(**inputs):
    from concourse.bass_utils import run_bass_kernel_spmd

    nc = _get_program()
    in_maps = _prep_in_maps(inputs)
    res = run_bass_kernel_spmd(nc, in_maps, core_ids=list(range(NCORES)))
    out = np.empty((B, T, DIM), dtype=np.float32)
    for b in range(B):
        out[b] = res.results[2 * b]["out_p"] + res.results[2 * b + 1]["out_p"]
    return out

